# revision 13
# baseline (speedup 1.0000x reference)
"""DenseQConv1D Trainium2 kernel.

Math: the reference computes, per output channel c and patch p (128-dim im2col
column of x, normalized):
    out[c,p] = sum_e sign(e) * (s_p^T (E @ R_c)[:128,:])_e^2
with R_c = kron of 9 RY(theta[c,q]) rotations and sign(e) = Z on the MSB qubit.
Because every RY factor is orthogonal and the measurement only touches qubit 0,
    R_c S R_c^T = kron([[cos t, sin t], [sin t, -cos t]], I_256),  t = theta[c,0]
so with E128 = E[:128,:], F = E128[:,:256], G = E128[:,256:]:
    GZ = F F^T - G G^T,  GX = F G^T + G F^T   (both 128x128, theta-independent)
    out[c,p] = (cos t_c * p^T GZ p + sin t_c * p^T GX p) / ||p||^2
This is exact for any entangle matrix / theta (validated: rel err ~9e-7).

Sharding: batch dimension across the 8 cores (core b computes x[b]); theta and
the entangle matrix are replicated. Everything (GZ/GX construction, im2col
quadratic forms, cos/sin, normalization) runs on-device.
"""

import math

import numpy as np

B = 8
C_IN = 16
C_OUT = 16
L = 1024
K = 8
L_OUT = L - K + 1  # 1017
LP = 1024  # padded patch count per core (cols 1017:1024 are dummy)
P = 128  # patch vector length = C_IN*K = partitions

_CACHE = {}


def _build_nc(dbg=False):
    import concourse.bacc as bacc
    import concourse.bass as bass
    import concourse.mybir as mybir
    import concourse.tile as tile
    from concourse import masks

    f32 = mybir.dt.float32
    nc = bacc.Bacc("TRN2", target_bir_lowering=False, debug=False)

    x_ext = nc.declare_dram_parameter("x", [C_IN, L], f32, isOutput=False)
    th_ext = nc.declare_dram_parameter("theta", [C_OUT, 9], f32, isOutput=False)
    e_ext = nc.declare_dram_parameter("entangle", [512, 512], f32, isOutput=False)
    out_ext = nc.declare_dram_parameter("out", [C_OUT, LP], f32, isOutput=True)
    dbg_ext = {}
    if dbg:
        for nm, shp in [
            ("d_pt", [P, LP]), ("d_e128", [P, 512]), ("d_csrow", [1, 2 * C_OUT]),
            ("d_csmat", [P, 2 * C_OUT]), ("d_gz", [P, P]), ("d_gx", [P, P]),
            ("d_n2all", [P, 8]), ("d_invr", [1, LP]), ("d_ptn", [P, LP]),
            ("d_mzn", [P, LP]), ("d_mxn", [P, LP]), ("d_et0", [P, P]),
        ]:
            dbg_ext[nm] = nc.declare_dram_parameter(nm, shp, f32, isOutput=True)

    with tile.TileContext(nc) as tc, tc.tile_pool(name="const", bufs=1) as const, \
            tc.tile_pool(name="sb", bufs=1) as sb, \
            tc.tile_pool(name="scr", bufs=2) as scrp:
        ident = const.tile([P, P], f32)
        masks.make_identity(nc, ident[:])
        ones1 = const.tile([1, P], f32)
        nc.vector.memset(ones1[:], 1.0)

        # ---- loads ----
        # PT[j*16+c, l] = x[c, l+j]  (im2col, row-permuted so each j is a
        # contiguous partition block; all row indices below live in this
        # permuted basis, applied consistently to e128 as well)
        pt = sb.tile([P, LP], f32)
        for j in range(K):
            nc.sync.dma_start(
                pt[16 * j : 16 * (j + 1), 0:L_OUT], x_ext[:, j : j + L_OUT]
            )
        nc.vector.memset(pt[:, L_OUT:LP], 1.0)

        # E rows 0..127 with the same (c j) -> (j c) row permutation
        e128 = sb.tile([P, 512], f32)
        e3d = e_ext[0:P, :].rearrange("(c j) f -> c j f", c=16, j=8)
        for j in range(K):
            nc.sync.dma_start(e128[16 * j : 16 * (j + 1), :], e3d[:, j, :])

        th = sb.tile([1, C_OUT], f32)
        nc.sync.dma_start(th[:], th_ext[:, 0:1].rearrange("p o -> o p"))

        # ---- cos/sin rows via half-angle of |t| (the ACT Sin PWP table is
        # only accurate for |x| <~ pi, so keep every Sin argument in range):
        #   a = |t|; u = sin(a/2); v = sin(pi/2 - a/2) = cos(a/2)
        #   cos t = 1 - 2u^2 ;  sin t = sign(t) * 2uv
        csrow = sb.tile([1, 2 * C_OUT], f32)
        AF = mybir.ActivationFunctionType
        bias_zero = const.tile([1, 1], f32)
        nc.scalar.memzero(bias_zero[:])
        ta = sb.tile([1, C_OUT], f32)
        nc.scalar.activation(ta[:], th[:], AF.Abs, bias=bias_zero[:])
        tsgn = sb.tile([1, C_OUT], f32)
        nc.scalar.activation(tsgn[:], th[:], AF.Sign, bias=bias_zero[:])
        u = sb.tile([1, C_OUT], f32)
        nc.scalar.activation(u[:], ta[:], AF.Sin, bias=bias_zero[:], scale=0.5)
        bias_half_pi = const.tile([1, 1], f32)
        nc.scalar.activation(
            bias_half_pi[:], bias_zero[:], AF.Copy, bias=math.pi / 2.0, scale=1.0
        )
        v = sb.tile([1, C_OUT], f32)
        nc.scalar.activation(v[:], ta[:], AF.Sin, bias=bias_half_pi[:], scale=-0.5)
        # cos t = -(sqrt(2) u)^2 + 1
        u2 = sb.tile([1, C_OUT], f32)
        nc.scalar.activation(
            u2[:], u[:], AF.Square, bias=bias_zero[:], scale=math.sqrt(2.0)
        )
        nc.scalar.activation(
            csrow[:, 0:C_OUT], u2[:], AF.Copy, bias=1.0, scale=-1.0
        )
        # sin t = sign(t) * 2 u v
        uv = sb.tile([1, C_OUT], f32)
        nc.vector.tensor_mul(uv[:], u[:], v[:])
        nc.vector.scalar_tensor_tensor(
            csrow[:, C_OUT : 2 * C_OUT], uv[:], 2.0, tsgn[:],
            op0=mybir.AluOpType.mult, op1=mybir.AluOpType.mult,
        )

        csmat = sb.tile([P, 2 * C_OUT], f32)
        et = [sb.tile([P, P], f32, name=f"et{k}", tag=f"et{k}") for k in range(4)]
        etn = [sb.tile([P, P], f32, name=f"etn{k}", tag=f"etn{k}") for k in range(2)]
        gz = sb.tile([P, P], f32)
        gx = sb.tile([P, P], f32)
        n2all = sb.tile([P, 8], f32)
        invcol = sb.tile([P, 8], f32)
        invr = sb.tile([1, LP], f32)

        # ======== PSUM phase 1: constants, Gram matrices, norms ========
        with tc.tile_pool(name="psA", bufs=2, space="PSUM") as psA, \
                tc.tile_pool(name="psG", bufs=2, space="PSUM") as psG:
            # cos/sin broadcast to 128 partitions via K=1 matmul
            csb_ps = psA.tile([P, 2 * C_OUT], f32, tag="eps")
            nc.tensor.matmul(csb_ps[:], ones1[:], csrow[:], start=True, stop=True)
            nc.scalar.copy(csmat[:], csb_ps[:])

            # E^T chunks (for the Gram matmuls)
            for k in range(4):
                etps = psA.tile([P, P], f32, tag="eps")
                nc.tensor.transpose(
                    etps[:], e128[:, 128 * k : 128 * (k + 1)], ident[:]
                )
                nc.scalar.copy(et[k][:], etps[:])
            for i, k in enumerate((2, 3)):
                nc.vector.tensor_scalar_mul(etn[i][:], et[k][:], -1.0)

            # GZ = F F^T - G G^T ; GX = F G^T + G F^T
            gzps = psG.tile([P, P], f32, tag="gram")
            nc.tensor.matmul(gzps[:], et[0][:], et[0][:], start=True, stop=False)
            nc.tensor.matmul(gzps[:], et[1][:], et[1][:], start=False, stop=False)
            nc.tensor.matmul(gzps[:], etn[0][:], et[2][:], start=False, stop=False)
            nc.tensor.matmul(gzps[:], etn[1][:], et[3][:], start=False, stop=True)
            nc.scalar.copy(gz[:], gzps[:])

            gxps = psG.tile([P, P], f32, tag="gram")
            nc.tensor.matmul(gxps[:], et[0][:], et[2][:], start=True, stop=False)
            nc.tensor.matmul(gxps[:], et[1][:], et[3][:], start=False, stop=False)
            nc.tensor.matmul(gxps[:], et[2][:], et[0][:], start=False, stop=False)
            nc.tensor.matmul(gxps[:], et[3][:], et[1][:], start=False, stop=True)
            nc.scalar.copy(gx[:], gxps[:])

            # squared norms per patch, in patch-on-partition layout
            for ch in range(8):
                p2ps = psA.tile([P, P], f32, tag="eps")
                nc.tensor.transpose(
                    p2ps[:], pt[:, 128 * ch : 128 * (ch + 1)], ident[:]
                )
                scr = scrp.tile([P, P], f32, tag="scr")
                nc.scalar.activation(
                    scr[:], p2ps[:], mybir.ActivationFunctionType.Square,
                    accum_out=n2all[:, ch : ch + 1],
                )
            nc.vector.tensor_scalar_max(n2all[:], n2all[:], 1e-24)
            nc.vector.reciprocal(invcol[:], n2all[:])
            # transpose each column to a (1, 128) row chunk -> (1, 1024) row
            invrps = psG.tile([1, LP], f32, tag="gram")
            for ch in range(8):
                nc.tensor.transpose(
                    invrps[:, 128 * ch : 128 * (ch + 1)], invcol[:, ch : ch + 1],
                    ident[:],
                )
            nc.scalar.copy(invr[:], invrps[:])

        # ======== PSUM phase 2: broadcast, quadratic forms, combine ========
        with tc.tile_pool(name="psB", bufs=1, space="PSUM") as psB:
            # inv broadcast to all 128 partitions: K=1 matmuls per 128-chunk
            invb = psB.tile([P, LP], f32, tag="invb")
            for ch in range(8):
                nc.tensor.matmul(
                    invb[:, 128 * ch : 128 * (ch + 1)], ones1[:],
                    invr[0:1, 128 * ch : 128 * (ch + 1)], start=True, stop=True,
                )

            # main quadratic forms
            ptn = sb.tile([P, LP], f32)
            nc.vector.tensor_mul(ptn[:], pt[:], invb[:])

            qz = psB.tile([P, LP], f32, tag="qz")
            qx = psB.tile([P, LP], f32, tag="qx")
            for h in range(2):
                s = slice(512 * h, 512 * (h + 1))
                nc.tensor.matmul(qz[:, s], gz[:], pt[:, s], start=True, stop=True)
            for h in range(2):
                s = slice(512 * h, 512 * (h + 1))
                nc.tensor.matmul(qx[:, s], gx[:], pt[:, s], start=True, stop=True)

            mzn = sb.tile([P, LP], f32)
            nc.vector.tensor_mul(mzn[:], ptn[:], qz[:])
            mxn = sb.tile([P, LP], f32)
            nc.vector.tensor_mul(mxn[:], ptn[:], qx[:])

            # channel combine: out[c,l] = cos_c*qZ'[l] + sin_c*qX'[l]
            out1 = psB.tile([C_OUT, LP], f32, tag="out1")
            for h in range(2):
                s = slice(512 * h, 512 * (h + 1))
                nc.tensor.matmul(
                    out1[:, s], csmat[:, 0:C_OUT], mzn[:, s],
                    start=True, stop=False,
                )
                nc.tensor.matmul(
                    out1[:, s], csmat[:, C_OUT : 2 * C_OUT], mxn[:, s],
                    start=False, stop=True,
                )

            outs = sb.tile([C_OUT, LP], f32)
            nc.scalar.copy(outs[:, 0:512], out1[:, 0:512])
            nc.vector.tensor_copy(outs[:, 512:LP], out1[:, 512:LP])
            nc.sync.dma_start(out_ext[:], outs[:])
            if dbg:
                for nm, t in [
                    ("d_pt", pt), ("d_e128", e128), ("d_csrow", csrow),
                    ("d_csmat", csmat), ("d_gz", gz), ("d_gx", gx),
                    ("d_n2all", n2all), ("d_invr", invr), ("d_ptn", ptn),
                    ("d_mzn", mzn), ("d_mxn", mxn), ("d_et0", et[0]),
                ]:
                    nc.sync.dma_start(dbg_ext[nm][:], t[:])

    nc.compile()
    return nc


def kernel(**inputs):
    from concourse.bass_utils import run_bass_kernel_spmd

    x = np.ascontiguousarray(np.asarray(inputs["x"], dtype=np.float32))
    theta = np.ascontiguousarray(np.asarray(inputs["theta"], dtype=np.float32))
    ent = np.ascontiguousarray(
        np.asarray(inputs["entangle_matrix"], dtype=np.float32)
    )

    if "nc" not in _CACHE:
        _CACHE["nc"] = _build_nc()
    nc = _CACHE["nc"]

    in_maps = [
        {"x": np.ascontiguousarray(x[b]), "theta": theta, "entangle": ent}
        for b in range(B)
    ]
    res = run_bass_kernel_spmd(nc, in_maps, core_ids=list(range(B)))
    out = np.stack([res.results[b]["out"][:, :L_OUT] for b in range(B)], axis=0)
    return np.ascontiguousarray(out.astype(np.float32))


# revision 14
# speedup vs baseline: 1.2393x; 1.2393x over previous
"""DenseQConv1D Trainium2 kernel.

Math: the reference computes, per output channel c and patch p (128-dim im2col
column of x, normalized):
    out[c,p] = sum_e sign(e) * (s_p^T (E @ R_c)[:128,:])_e^2
with R_c = kron of 9 RY(theta[c,q]) rotations and sign(e) = Z on the MSB qubit.
Because every RY factor is orthogonal and the measurement only touches qubit 0,
    R_c S R_c^T = kron([[cos t, sin t], [sin t, -cos t]], I_256),  t = theta[c,0]
so with E128 = E[:128,:], F = E128[:,:256], G = E128[:,256:]:
    GZ = F F^T - G G^T,  GX = F G^T + G F^T   (both 128x128, theta-independent)
    out[c,p] = (cos t_c * p^T GZ p + sin t_c * p^T GX p) / ||p||^2
This is exact for any entangle matrix / theta (validated: rel err ~9e-7).

Sharding: batch dimension across the 8 cores (core b computes x[b]); theta and
the entangle matrix are replicated. Everything (GZ/GX construction, im2col
quadratic forms, cos/sin, normalization) runs on-device.
"""

import math

import numpy as np

B = 8
C_IN = 16
C_OUT = 16
L = 1024
K = 8
L_OUT = L - K + 1  # 1017
LP = 1024  # padded patch count per core (cols 1017:1024 are dummy)
P = 128  # patch vector length = C_IN*K = partitions

_CACHE = {}


def _build_nc(dbg=False):
    import bass_rust as _br
    import concourse.bacc as bacc
    import concourse.mybir as mybir
    import concourse.tile as tile
    from concourse import masks

    f32 = mybir.dt.float32
    nc = bacc.Bacc("TRN2", target_bir_lowering=False, debug=False)

    x_ext = nc.declare_dram_parameter("x", [C_IN, L], f32, isOutput=False)
    th_ext = nc.declare_dram_parameter("theta", [C_OUT, 9], f32, isOutput=False)
    e_ext = nc.declare_dram_parameter("entangle", [512, 512], f32, isOutput=False)
    out_ext = nc.declare_dram_parameter("out", [C_OUT, LP], f32, isOutput=True)
    inv_dram = nc.dram_tensor("inv_scratch", [1, LP], f32)
    dbg_ext = {}
    if dbg:
        for nm, shp in [
            ("d_pt", [P, LP]), ("d_e128", [P, 512]), ("d_csrow", [1, 2 * C_OUT]),
            ("d_csmat", [P, 2 * C_OUT]), ("d_gz", [P, P]), ("d_gx", [P, P]),
            ("d_n2all", [P, 8]), ("d_invr", [1, LP]), ("d_invb", [C_OUT, LP]),
            ("d_mzn", [P, LP]), ("d_mxn", [P, LP]), ("d_et0", [P, P]),
        ]:
            dbg_ext[nm] = nc.declare_dram_parameter(nm, shp, f32, isOutput=True)

    with tile.TileContext(nc) as tc, tc.tile_pool(name="const", bufs=1) as const, \
            tc.tile_pool(name="sb", bufs=1) as sb, \
            tc.tile_pool(name="scr", bufs=2) as scrp:
        ident = const.tile([P, P], f32)
        masks.make_identity(nc, ident[:])
        ones1 = const.tile([1, P], f32)
        nc.vector.memset(ones1[:], 1.0)

        # ---- loads ----
        # PT[j*16+c, l] = x[c, l+j]: single im2col DMA with an overlapping
        # window access pattern (j outer / c inner on partitions, matching the
        # row permutation used for e128 below)
        pt = sb.tile([P, LP], f32)
        x_win = _br.AP(x_ext, 0, [[1, K], [L, C_IN], [1, L_OUT]])
        nc.sync.dma_start(pt[:, 0:L_OUT], x_win)
        nc.vector.memset(pt[:, L_OUT:LP], 1.0)

        # E rows 0..127 with the same (c j) -> (j c) row permutation, on the
        # ACT HWDGE ring so it runs parallel to the x load
        e128 = sb.tile([P, 512], f32)
        e_perm = _br.AP(e_ext, 0, [[512, K], [512 * K, C_IN], [1, 512]])
        nc.scalar.dma_start(e128[:], e_perm)

        th = sb.tile([1, C_OUT], f32)
        nc.sync.dma_start(th[:], th_ext[:, 0:1].rearrange("p o -> o p"))

        # ---- cos/sin rows via half-angle of |t| (the ACT Sin PWP table is
        # only accurate for |x| <~ pi, so keep every Sin argument in range):
        #   a = |t|; u = sin(a/2); v = sin(pi/2 - a/2) = cos(a/2)
        #   cos t = 1 - 2u^2 ;  sin t = sign(t) * 2uv
        csrow = sb.tile([1, 2 * C_OUT], f32)
        AF = mybir.ActivationFunctionType
        bias_zero = const.tile([1, 1], f32)
        nc.scalar.memzero(bias_zero[:])
        ta = sb.tile([1, C_OUT], f32)
        nc.scalar.activation(ta[:], th[:], AF.Abs, bias=bias_zero[:])
        tsgn = sb.tile([1, C_OUT], f32)
        nc.scalar.activation(tsgn[:], th[:], AF.Sign, bias=bias_zero[:])
        u = sb.tile([1, C_OUT], f32)
        nc.scalar.activation(u[:], ta[:], AF.Sin, bias=bias_zero[:], scale=0.5)
        bias_half_pi = const.tile([1, 1], f32)
        nc.scalar.activation(
            bias_half_pi[:], bias_zero[:], AF.Copy, bias=math.pi / 2.0, scale=1.0
        )
        v = sb.tile([1, C_OUT], f32)
        nc.scalar.activation(v[:], ta[:], AF.Sin, bias=bias_half_pi[:], scale=-0.5)
        u2 = sb.tile([1, C_OUT], f32)
        nc.scalar.activation(
            u2[:], u[:], AF.Square, bias=bias_zero[:], scale=math.sqrt(2.0)
        )
        nc.scalar.activation(
            csrow[:, 0:C_OUT], u2[:], AF.Copy, bias=1.0, scale=-1.0
        )
        uv = sb.tile([1, C_OUT], f32)
        nc.vector.tensor_mul(uv[:], u[:], v[:])
        nc.vector.scalar_tensor_tensor(
            csrow[:, C_OUT : 2 * C_OUT], uv[:], 2.0, tsgn[:],
            op0=mybir.AluOpType.mult, op1=mybir.AluOpType.mult,
        )

        csmat = sb.tile([P, 2 * C_OUT], f32)
        et = [sb.tile([P, P], f32, name=f"et{k}", tag=f"et{k}") for k in range(4)]
        etn = [sb.tile([P, P], f32, name=f"etn{k}", tag=f"etn{k}") for k in range(2)]
        gz = sb.tile([P, P], f32)
        gx = sb.tile([P, P], f32)
        n2all = sb.tile([P, 8], f32)
        invcol = sb.tile([P, 8], f32)
        invr = sb.tile([1, LP], f32)
        invb = sb.tile([C_OUT, LP], f32)

        # ======== PSUM phase 1: constants, Gram matrices, norms ========
        with tc.tile_pool(name="psA", bufs=2, space="PSUM") as psA, \
                tc.tile_pool(name="psG", bufs=2, space="PSUM") as psG:
            # cos/sin broadcast to 128 partitions via K=1 matmul
            csb_ps = psA.tile([P, 2 * C_OUT], f32, tag="eps")
            nc.tensor.matmul(csb_ps[:], ones1[:], csrow[:], start=True, stop=True)
            nc.scalar.copy(csmat[:], csb_ps[:])

            # E^T chunks (for the Gram matmuls)
            for k in range(4):
                etps = psA.tile([P, P], f32, tag="eps")
                nc.tensor.transpose(
                    etps[:], e128[:, 128 * k : 128 * (k + 1)], ident[:]
                )
                nc.scalar.copy(et[k][:], etps[:])
            for i, k in enumerate((2, 3)):
                nc.vector.tensor_scalar_mul(etn[i][:], et[k][:], -1.0)

            # GZ = F F^T - G G^T ; GX = F G^T + G F^T
            gzps = psG.tile([P, P], f32, tag="gram")
            nc.tensor.matmul(gzps[:], et[0][:], et[0][:], start=True, stop=False)
            nc.tensor.matmul(gzps[:], et[1][:], et[1][:], start=False, stop=False)
            nc.tensor.matmul(gzps[:], etn[0][:], et[2][:], start=False, stop=False)
            nc.tensor.matmul(gzps[:], etn[1][:], et[3][:], start=False, stop=True)
            nc.scalar.copy(gz[:], gzps[:])

            gxps = psG.tile([P, P], f32, tag="gram")
            nc.tensor.matmul(gxps[:], et[0][:], et[2][:], start=True, stop=False)
            nc.tensor.matmul(gxps[:], et[1][:], et[3][:], start=False, stop=False)
            nc.tensor.matmul(gxps[:], et[2][:], et[0][:], start=False, stop=False)
            nc.tensor.matmul(gxps[:], et[3][:], et[1][:], start=False, stop=True)
            nc.scalar.copy(gx[:], gxps[:])

            # squared norms per patch, in patch-on-partition layout
            for ch in range(8):
                p2ps = psA.tile([P, P], f32, tag="eps")
                nc.tensor.transpose(
                    p2ps[:], pt[:, 128 * ch : 128 * (ch + 1)], ident[:]
                )
                scr = scrp.tile([P, P], f32, tag="scr")
                nc.scalar.activation(
                    scr[:], p2ps[:], mybir.ActivationFunctionType.Square,
                    accum_out=n2all[:, ch : ch + 1],
                )
            nc.vector.tensor_scalar_max(n2all[:], n2all[:], 1e-24)
            nc.vector.reciprocal(invcol[:], n2all[:])
            # transpose each column to a (1, 128) row chunk -> (1, 1024) row
            invrps = psG.tile([1, LP], f32, tag="gram")
            for ch in range(8):
                nc.tensor.transpose(
                    invrps[:, 128 * ch : 128 * (ch + 1)], invcol[:, ch : ch + 1],
                    ident[:],
                )
            nc.scalar.copy(invr[:], invrps[:])
            # broadcast 1/n2 to the 16 output-channel partitions via a DRAM
            # bounce (0-stride partition reads are only legal on DRAM APs)
            nc.scalar.dma_start(inv_dram[:], invr[:])
            inv_rep = _br.AP(inv_dram, 0, [[0, C_OUT], [1, LP]])
            nc.scalar.dma_start(invb[:], inv_rep)

        # ======== PSUM phase 2: quadratic forms + channel combine ========
        with tc.tile_pool(name="psB", bufs=1, space="PSUM") as psB:
            qz = psB.tile([P, LP], f32, tag="qz")
            qx = psB.tile([P, LP], f32, tag="qx")
            for h in range(2):
                s = slice(512 * h, 512 * (h + 1))
                nc.tensor.matmul(qz[:, s], gz[:], pt[:, s], start=True, stop=True)
            for h in range(2):
                s = slice(512 * h, 512 * (h + 1))
                nc.tensor.matmul(qx[:, s], gx[:], pt[:, s], start=True, stop=True)

            mzn = sb.tile([P, LP], f32)
            nc.vector.tensor_mul(mzn[:], pt[:], qz[:])
            mxn = sb.tile([P, LP], f32)
            nc.vector.tensor_mul(mxn[:], pt[:], qx[:])

            # channel combine: out1[c,l] = cos_c*qZ[l] + sin_c*qX[l]
            out1 = psB.tile([C_OUT, LP], f32, tag="out1")
            for h in range(2):
                s = slice(512 * h, 512 * (h + 1))
                nc.tensor.matmul(
                    out1[:, s], csmat[:, 0:C_OUT], mzn[:, s],
                    start=True, stop=False,
                )
                nc.tensor.matmul(
                    out1[:, s], csmat[:, C_OUT : 2 * C_OUT], mxn[:, s],
                    start=False, stop=True,
                )

            # divide by ||p||^2 while evacuating PSUM
            outs = sb.tile([C_OUT, LP], f32)
            nc.vector.tensor_mul(outs[:], invb[:], out1[:])
            nc.sync.dma_start(out_ext[:], outs[:])
            if dbg:
                for nm, t in [
                    ("d_pt", pt), ("d_e128", e128), ("d_csrow", csrow),
                    ("d_csmat", csmat), ("d_gz", gz), ("d_gx", gx),
                    ("d_n2all", n2all), ("d_invr", invr), ("d_invb", invb),
                    ("d_mzn", mzn), ("d_mxn", mxn), ("d_et0", et[0]),
                ]:
                    nc.sync.dma_start(dbg_ext[nm][:], t[:])

    nc.compile()
    return nc


def kernel(**inputs):
    from concourse.bass_utils import run_bass_kernel_spmd

    x = np.ascontiguousarray(np.asarray(inputs["x"], dtype=np.float32))
    theta = np.ascontiguousarray(np.asarray(inputs["theta"], dtype=np.float32))
    ent = np.ascontiguousarray(
        np.asarray(inputs["entangle_matrix"], dtype=np.float32)
    )

    if "nc" not in _CACHE:
        _CACHE["nc"] = _build_nc()
    nc = _CACHE["nc"]

    in_maps = [
        {"x": np.ascontiguousarray(x[b]), "theta": theta, "entangle": ent}
        for b in range(B)
    ]
    res = run_bass_kernel_spmd(nc, in_maps, core_ids=list(range(B)))
    out = np.stack([res.results[b]["out"][:, :L_OUT] for b in range(B)], axis=0)
    return np.ascontiguousarray(out.astype(np.float32))


# revision 21
# speedup vs baseline: 1.4139x; 1.1409x over previous
"""DenseQConv1D Trainium2 kernel.

Math: the reference computes, per output channel c and patch p (128-dim im2col
column of x, normalized):
    out[c,p] = sum_e sign(e) * (s_p^T (E @ R_c)[:128,:])_e^2
with R_c = kron of 9 RY(theta[c,q]) rotations and sign(e) = Z on the MSB qubit.
Because every RY factor is orthogonal and the measurement only touches qubit 0,
    R_c S R_c^T = kron([[cos t, sin t], [sin t, -cos t]], I_256),  t = theta[c,0]
so with E128 = E[:128,:], F = E128[:,:256], G = E128[:,256:]:
    GZ = F F^T - G G^T,  GX = F G^T + G F^T   (both 128x128, theta-independent)
    out[c,p] = (cos t_c * p^T GZ p + sin t_c * p^T GX p) / ||p||^2
This is exact for any entangle matrix / theta (validated: rel err ~9e-7 in
fp32; the shipped kernel uses fp32r matmuls, rel err ~1e-4).

Sharding: batch dimension across the 8 cores (core b computes x[b]); theta and
the entangle matrix are replicated. Everything (GZ/GX construction, im2col
quadratic forms, cos/sin, normalization) runs on-device.
"""

import math
from contextlib import ExitStack as _ExitStack

import numpy as np

B = 8
C_IN = 16
C_OUT = 16
L = 1024
K = 8
L_OUT = L - K + 1  # 1017
LP = 1024  # padded patch count per core (cols 1017:1024 are dummy)
P = 128  # patch vector length = C_IN*K = partitions

_CACHE = {}


def _build_nc(dbg=False):
    import bass_rust as _br
    import concourse.bacc as bacc
    import concourse.mybir as mybir
    import concourse.tile as tile
    from concourse import masks

    f32 = mybir.dt.float32
    f32r = mybir.dt.float32r
    AF = mybir.ActivationFunctionType
    ALU = mybir.AluOpType
    nc = bacc.Bacc("TRN2", target_bir_lowering=False, debug=False)

    x_ext = nc.declare_dram_parameter("x", [C_IN, L], f32, isOutput=False)
    th_ext = nc.declare_dram_parameter("theta", [C_OUT, 9], f32, isOutput=False)
    e_ext = nc.declare_dram_parameter("entangle", [512, 512], f32, isOutput=False)
    out_ext = nc.declare_dram_parameter("out", [C_OUT, LP], f32, isOutput=True)
    inv_dram = nc.dram_tensor("inv_scratch", [1, LP], f32)
    dbg_ext = {}
    if dbg:
        for nm, shp in [
            ("d_pt", [P, LP]), ("d_e128", [P, 512]), ("d_csrow", [1, 2 * C_OUT]),
            ("d_gz", [P, P]), ("d_gx", [P, P]),
            ("d_n2all", [P, 8]), ("d_invb", [C_OUT, LP]),
            ("d_mzn", [P, LP]), ("d_mxn", [P, LP]),
        ]:
            dbg_ext[nm] = nc.declare_dram_parameter(nm, shp, f32, isOutput=True)

    with tile.TileContext(nc) as tc, tc.tile_pool(name="const", bufs=1) as const, \
            tc.tile_pool(name="sb", bufs=1) as sb, \
            tc.tile_pool(name="scr", bufs=2) as scrp:
        ident = const.tile([P, P], f32)
        masks.make_identity(nc, ident[:])
        ones1 = const.tile([1, P], f32)
        nc.vector.memset(ones1[:], 1.0)

        # ---- loads, spread over three DMA paths ----
        # theta first (tiny) so the trig chain is never queued behind x
        th = sb.tile([1, C_OUT], f32)
        nc.sync.dma_start(th[:], th_ext[:, 0:1].rearrange("p o -> o p"))

        # PT[j*16+c, l] = x[c, l+j]: im2col via overlapping-window APs,
        # split into two halves on the two HWDGE rings
        pt = sb.tile([P, LP], f32)
        x_win0 = _br.AP(x_ext, 0, [[1, 4], [L, C_IN], [1, L_OUT]])
        x_win1 = _br.AP(x_ext, 4, [[1, 4], [L, C_IN], [1, L_OUT]])
        nc.sync.dma_start(pt[0:64, 0:L_OUT], x_win0)
        nc.scalar.dma_start(pt[64:P, 0:L_OUT], x_win1)
        nc.vector.memset(pt[:, L_OUT:LP], 1.0)

        # E rows 0..127, row-permuted (c j) -> (j c), on the SWDGE path
        e128 = sb.tile([P, 512], f32)
        e_perm = _br.AP(e_ext, 0, [[512, K], [512 * K, C_IN], [1, 512]])
        nc.gpsimd.dma_start(e128[:], e_perm)

        # ---- cos/sin rows via half-angle of |t| (the ACT Sin PWP table is
        # only accurate for |x| <~ pi):
        #   a = |t|; u = sin(a/2); v = cos(a/2)
        #   cos t = 1 - 2u^2 ;  sin t = sign(t) * 2uv
        csrow = sb.tile([1, 2 * C_OUT], f32r)
        bias_zero = const.tile([1, 1], f32)
        nc.scalar.memzero(bias_zero[:])
        bias_half_pi = const.tile([1, 1], f32)
        nc.scalar.activation(
            bias_half_pi[:], bias_zero[:], AF.Copy, bias=math.pi / 2.0, scale=1.0
        )
        ta = sb.tile([1, C_OUT], f32)
        nc.vector.scalar_tensor_tensor(
            ta[:], th[:], -1.0, th[:], op0=ALU.mult, op1=ALU.max
        )
        tsgn = sb.tile([1, C_OUT], f32)
        nc.vector.tensor_scalar(tsgn[:], th[:], 0.0, None, op0=ALU.is_gt)
        nc.vector.tensor_scalar(
            tsgn[:], tsgn[:], 2.0, 1.0, op0=ALU.mult, op1=ALU.subtract
        )
        u = sb.tile([1, C_OUT], f32)
        nc.scalar.activation(u[:], ta[:], AF.Sin, bias=bias_zero[:], scale=0.5)
        v = sb.tile([1, C_OUT], f32)
        nc.scalar.activation(v[:], ta[:], AF.Sin, bias=bias_half_pi[:], scale=-0.5)
        u2 = sb.tile([1, C_OUT], f32)
        nc.scalar.activation(
            u2[:], u[:], AF.Square, bias=bias_zero[:], scale=math.sqrt(2.0)
        )
        nc.scalar.activation(
            csrow[:, 0:C_OUT], u2[:], AF.Copy, bias=1.0, scale=-1.0
        )
        uv = sb.tile([1, C_OUT], f32)
        nc.vector.tensor_mul(uv[:], u[:], v[:])
        nc.vector.scalar_tensor_tensor(
            csrow[:, C_OUT : 2 * C_OUT], uv[:], 2.0, tsgn[:],
            op0=ALU.mult, op1=ALU.mult,
        )

        # fp32r copies of pt for the PE (producers must round for fp32r mms)
        ptr = sb.tile([P, LP], f32r)
        nc.scalar.copy(ptr[:], pt[:])

        csmat = sb.tile([P, 2 * C_OUT], f32r)
        et = [sb.tile([P, P], f32r, name=f"et{k}", tag=f"et{k}") for k in range(4)]
        etn = [sb.tile([P, P], f32r, name=f"etn{k}", tag=f"etn{k}") for k in range(2)]
        gz = sb.tile([P, P], f32r)
        gx = sb.tile([P, P], f32r)
        n2all = sb.tile([P, 8], f32)
        invcol = sb.tile([P, 8], f32)
        invt8 = sb.tile([8, P], f32)
        invb = sb.tile([C_OUT, LP], f32)

        # PSUM pools in strict stack order: psA+psG (4 banks, outer, live
        # throughout), psB (qz/qx, 4 banks) closed before psC (out1) opens.
        with _ExitStack() as ps_stack:
            psA = ps_stack.enter_context(
                tc.tile_pool(name="psA", bufs=2, space="PSUM")
            )
            psG = ps_stack.enter_context(
                tc.tile_pool(name="psG", bufs=2, space="PSUM")
            )
            psB_cm = tc.tile_pool(name="psB", bufs=1, space="PSUM")
            psB = psB_cm.__enter__()

            # E^T chunks (PE order: these first — only need e128 + ident)
            for k in range(4):
                etps = psA.tile([P, P], f32, tag="eps")
                nc.tensor.transpose(
                    etps[:], e128[:, 128 * k : 128 * (k + 1)], ident[:]
                )
                nc.scalar.copy(et[k][:], etps[:])
            for i, k in enumerate((2, 3)):
                nc.vector.tensor_scalar_mul(etn[i][:], et[k][:], -1.0)

            # GZ = F F^T - G G^T ; GX = F G^T + G F^T
            gzps = psG.tile([P, P], f32, tag="gram")
            nc.tensor.matmul(gzps[:], et[0][:], et[0][:], start=True, stop=False)
            nc.tensor.matmul(gzps[:], et[1][:], et[1][:], start=False, stop=False)
            nc.tensor.matmul(gzps[:], etn[0][:], et[2][:], start=False, stop=False)
            nc.tensor.matmul(gzps[:], etn[1][:], et[3][:], start=False, stop=True)
            nc.scalar.copy(gz[:], gzps[:])

            gxps = psG.tile([P, P], f32, tag="gram")
            nc.tensor.matmul(gxps[:], et[0][:], et[2][:], start=True, stop=False)
            nc.tensor.matmul(gxps[:], et[1][:], et[3][:], start=False, stop=False)
            nc.tensor.matmul(gxps[:], et[2][:], et[0][:], start=False, stop=False)
            nc.tensor.matmul(gxps[:], et[3][:], et[1][:], start=False, stop=True)
            nc.scalar.copy(gx[:], gxps[:])

            # squared norms per patch: transpose 128-patch blocks, square on
            # ACT (evac) + multiply-reduce on DVE
            for ch in range(8):
                p2ps = psA.tile([P, P], f32, tag="eps")
                nc.tensor.transpose(
                    p2ps[:], pt[:, 128 * ch : 128 * (ch + 1)], ident[:]
                )
                scr = scrp.tile([P, P], f32, tag="scr")
                nc.scalar.activation(
                    scr[:], p2ps[:], AF.Square,
                    accum_out=n2all[:, ch : ch + 1],
                )

            # main quadratic forms (start as soon as gz/gx + ptr are ready)
            qz = psB.tile([P, LP], f32, tag="qz")
            qx = psB.tile([P, LP], f32, tag="qx")
            for h in range(2):
                s = slice(512 * h, 512 * (h + 1))
                nc.tensor.matmul(qz[:, s], gz[:], ptr[:, s], start=True, stop=True)
            for h in range(2):
                s = slice(512 * h, 512 * (h + 1))
                nc.tensor.matmul(qx[:, s], gx[:], ptr[:, s], start=True, stop=True)

            mzn = sb.tile([P, LP], f32r)
            nc.vector.tensor_mul(mzn[:], pt[:], qz[:])
            mxn = sb.tile([P, LP], f32r)
            nc.vector.tensor_mul(mxn[:], pt[:], qx[:])

            # 1/||p||^2 -> single transpose -> DRAM bounce broadcast to 16 rows
            nc.vector.tensor_scalar_max(n2all[:], n2all[:], 1e-24)
            nc.vector.reciprocal(invcol[:], n2all[:])
            invt8ps = psG.tile([8, P], f32, tag="gram")
            nc.tensor.transpose(invt8ps[:], invcol[:], ident[:])
            nc.scalar.copy(invt8[:], invt8ps[:])
            nc.scalar.dma_start(
                inv_dram[:].rearrange("o (c p) -> (o c) p", c=8, p=P), invt8[:]
            )
            inv_rep = _br.AP(inv_dram, 0, [[0, C_OUT], [1, LP]])
            nc.scalar.dma_start(invb[:], inv_rep)

            # cos/sin broadcast to 128 partitions via K=1 matmul (late on PE)
            csb_ps = psA.tile([P, 2 * C_OUT], f32, tag="eps")
            nc.tensor.matmul(
                csb_ps[:], ones1[:].bitcast(f32r), csrow[:], start=True, stop=True
            )
            nc.scalar.copy(csmat[:], csb_ps[:])

            psB_cm.__exit__(None, None, None)

            # channel combine: out1[c,l] = cos_c*qZ[l] + sin_c*qX[l]
            psC = ps_stack.enter_context(
                tc.tile_pool(name="psC", bufs=1, space="PSUM")
            )
            out1 = psC.tile([C_OUT, LP], f32, tag="out1")
            outs = sb.tile([C_OUT, LP], f32)
            for h in range(2):
                s = slice(512 * h, 512 * (h + 1))
                nc.tensor.matmul(
                    out1[:, s], csmat[:, 0:C_OUT], mzn[:, s],
                    start=True, stop=False,
                )
                nc.tensor.matmul(
                    out1[:, s], csmat[:, C_OUT : 2 * C_OUT], mxn[:, s],
                    start=False, stop=True,
                )
                # divide by ||p||^2 while evacuating PSUM, pipelined per half
                nc.vector.tensor_mul(outs[:, s], invb[:, s], out1[:, s])
                nc.sync.dma_start(out_ext[:, s], outs[:, s])

            if dbg:
                for nm, t in [
                    ("d_pt", pt), ("d_e128", e128), ("d_csrow", csrow),
                    ("d_gz", gz), ("d_gx", gx), ("d_n2all", n2all),
                    ("d_invb", invb), ("d_mzn", mzn), ("d_mxn", mxn),
                ]:
                    nc.sync.dma_start(
                        dbg_ext[nm][:], t[:].bitcast(f32)
                    )


    nc.compile()
    return nc


def kernel(**inputs):
    from concourse.bass_utils import run_bass_kernel_spmd

    x = np.ascontiguousarray(np.asarray(inputs["x"], dtype=np.float32))
    theta = np.ascontiguousarray(np.asarray(inputs["theta"], dtype=np.float32))
    ent = np.ascontiguousarray(
        np.asarray(inputs["entangle_matrix"], dtype=np.float32)
    )

    if "nc" not in _CACHE:
        _CACHE["nc"] = _build_nc()
    nc = _CACHE["nc"]

    in_maps = [
        {"x": np.ascontiguousarray(x[b]), "theta": theta, "entangle": ent}
        for b in range(B)
    ]
    res = run_bass_kernel_spmd(nc, in_maps, core_ids=list(range(B)))
    out = np.stack([res.results[b]["out"][:, :L_OUT] for b in range(B)], axis=0)
    return np.ascontiguousarray(out.astype(np.float32))


# revision 22
# speedup vs baseline: 1.4290x; 1.0107x over previous
"""DenseQConv1D Trainium2 kernel.

Math: the reference computes, per output channel c and patch p (128-dim im2col
column of x, normalized):
    out[c,p] = sum_e sign(e) * (s_p^T (E @ R_c)[:128,:])_e^2
with R_c = kron of 9 RY(theta[c,q]) rotations and sign(e) = Z on the MSB qubit.
Because every RY factor is orthogonal and the measurement only touches qubit 0,
    R_c S R_c^T = kron([[cos t, sin t], [sin t, -cos t]], I_256),  t = theta[c,0]
so with E128 = E[:128,:], F = E128[:,:256], G = E128[:,256:]:
    GZ = F F^T - G G^T,  GX = F G^T + G F^T   (both 128x128, theta-independent)
    out[c,p] = (cos t_c * p^T GZ p + sin t_c * p^T GX p) / ||p||^2
This is exact for any entangle matrix / theta (validated: rel err ~9e-7 in
fp32; the shipped kernel uses fp32r matmuls, rel err ~1e-4).

Sharding: batch dimension across the 8 cores (core b computes x[b]); theta and
the entangle matrix are replicated. Everything (GZ/GX construction, im2col
quadratic forms, cos/sin, normalization) runs on-device.
"""

import math
from contextlib import ExitStack as _ExitStack

import numpy as np

B = 8
C_IN = 16
C_OUT = 16
L = 1024
K = 8
L_OUT = L - K + 1  # 1017
LP = 1024  # padded patch count per core (cols 1017:1024 are dummy)
P = 128  # patch vector length = C_IN*K = partitions

_CACHE = {}


def _build_nc(dbg=False):
    import bass_rust as _br
    import concourse.bacc as bacc
    import concourse.mybir as mybir
    import concourse.tile as tile
    from concourse import masks

    f32 = mybir.dt.float32
    f32r = mybir.dt.float32r
    AF = mybir.ActivationFunctionType
    ALU = mybir.AluOpType
    nc = bacc.Bacc("TRN2", target_bir_lowering=False, debug=False)

    x_ext = nc.declare_dram_parameter("x", [C_IN, L], f32, isOutput=False)
    th_ext = nc.declare_dram_parameter("theta", [C_OUT, 9], f32, isOutput=False)
    e_ext = nc.declare_dram_parameter("entangle", [512, 512], f32, isOutput=False)
    out_ext = nc.declare_dram_parameter("out", [C_OUT, LP], f32, isOutput=True)
    inv_dram = nc.dram_tensor("inv_scratch", [1, LP], f32)
    dbg_ext = {}
    if dbg:
        for nm, shp in [
            ("d_pt", [P, LP]), ("d_e128", [P, 512]), ("d_csrow", [1, 2 * C_OUT]),
            ("d_gz", [P, P]), ("d_gx", [P, P]),
            ("d_n2all", [P, 8]), ("d_invb", [C_OUT, LP]),
            ("d_mzn", [P, LP]), ("d_mxn", [P, LP]),
        ]:
            dbg_ext[nm] = nc.declare_dram_parameter(nm, shp, f32, isOutput=True)

    with tile.TileContext(nc) as tc, tc.tile_pool(name="const", bufs=1) as const, \
            tc.tile_pool(name="sb", bufs=1) as sb, \
            tc.tile_pool(name="scr", bufs=2) as scrp:
        ident = const.tile([P, P], f32)
        masks.make_identity(nc, ident[:])
        ones1 = const.tile([1, P], f32)
        nc.vector.memset(ones1[:], 1.0)

        # ---- loads: theta (tiny) first, then E halves, then x halves;
        # interleaved across the two HWDGE rings so E lands first ----
        th = sb.tile([1, C_OUT], f32)
        nc.sync.dma_start(th[:], th_ext[:, 0:1].rearrange("p o -> o p"))

        # E rows 0..127, row-permuted (c j) -> (j c); partition halves on
        # the two rings
        e128 = sb.tile([P, 512], f32)
        e_perm0 = _br.AP(e_ext, 0, [[512, 4], [512 * K, C_IN], [1, 512]])
        e_perm1 = _br.AP(e_ext, 4 * 512, [[512, 4], [512 * K, C_IN], [1, 512]])
        nc.sync.dma_start(e128[0:64, :], e_perm0)
        nc.scalar.dma_start(e128[64:P, :], e_perm1)

        # PT[j*16+c, l] = x[c, l+j]: im2col via overlapping-window APs
        pt = sb.tile([P, LP], f32)
        x_win0 = _br.AP(x_ext, 0, [[1, 4], [L, C_IN], [1, L_OUT]])
        x_win1 = _br.AP(x_ext, 4, [[1, 4], [L, C_IN], [1, L_OUT]])
        nc.sync.dma_start(pt[0:64, 0:L_OUT], x_win0)
        nc.scalar.dma_start(pt[64:P, 0:L_OUT], x_win1)
        nc.vector.memset(pt[:, L_OUT:LP], 1.0)

        # ---- cos/sin rows via half-angle of |t| (the ACT Sin PWP table is
        # only accurate for |x| <~ pi):
        #   a = |t|; u = sin(a/2); v = cos(a/2)
        #   cos t = 1 - 2u^2 ;  sin t = sign(t) * 2uv
        csrow = sb.tile([1, 2 * C_OUT], f32r)
        bias_zero = const.tile([1, 1], f32)
        nc.scalar.memzero(bias_zero[:])
        bias_half_pi = const.tile([1, 1], f32)
        nc.scalar.activation(
            bias_half_pi[:], bias_zero[:], AF.Copy, bias=math.pi / 2.0, scale=1.0
        )
        ta = sb.tile([1, C_OUT], f32)
        nc.vector.scalar_tensor_tensor(
            ta[:], th[:], -1.0, th[:], op0=ALU.mult, op1=ALU.max
        )
        tsgn = sb.tile([1, C_OUT], f32)
        nc.vector.tensor_scalar(tsgn[:], th[:], 0.0, None, op0=ALU.is_gt)
        nc.vector.tensor_scalar(
            tsgn[:], tsgn[:], 2.0, 1.0, op0=ALU.mult, op1=ALU.subtract
        )
        u = sb.tile([1, C_OUT], f32)
        nc.scalar.activation(u[:], ta[:], AF.Sin, bias=bias_zero[:], scale=0.5)
        v = sb.tile([1, C_OUT], f32)
        nc.scalar.activation(v[:], ta[:], AF.Sin, bias=bias_half_pi[:], scale=-0.5)
        u2 = sb.tile([1, C_OUT], f32)
        nc.scalar.activation(
            u2[:], u[:], AF.Square, bias=bias_zero[:], scale=math.sqrt(2.0)
        )
        nc.scalar.activation(
            csrow[:, 0:C_OUT], u2[:], AF.Copy, bias=1.0, scale=-1.0
        )
        uv = sb.tile([1, C_OUT], f32)
        nc.vector.tensor_mul(uv[:], u[:], v[:])
        nc.vector.scalar_tensor_tensor(
            csrow[:, C_OUT : 2 * C_OUT], uv[:], 2.0, tsgn[:],
            op0=ALU.mult, op1=ALU.mult,
        )

        # fp32r copies of pt for the PE (producers must round for fp32r mms)
        ptr = sb.tile([P, LP], f32r)
        nc.scalar.copy(ptr[:], pt[:])

        csmat = sb.tile([P, 2 * C_OUT], f32r)
        et = [sb.tile([P, P], f32r, name=f"et{k}", tag=f"et{k}") for k in range(4)]
        etn = [sb.tile([P, P], f32r, name=f"etn{k}", tag=f"etn{k}") for k in range(2)]
        gz = sb.tile([P, P], f32r)
        gx = sb.tile([P, P], f32r)
        n2all = sb.tile([P, 8], f32)
        invcol = sb.tile([P, 8], f32)
        invt8 = sb.tile([8, P], f32)
        invb = sb.tile([C_OUT, LP], f32)

        # PSUM pools in strict stack order: psA+psG (4 banks, outer, live
        # throughout), psB (qz/qx, 4 banks) closed before psC (out1) opens.
        with _ExitStack() as ps_stack:
            psA = ps_stack.enter_context(
                tc.tile_pool(name="psA", bufs=2, space="PSUM")
            )
            psG = ps_stack.enter_context(
                tc.tile_pool(name="psG", bufs=2, space="PSUM")
            )
            psB_cm = tc.tile_pool(name="psB", bufs=1, space="PSUM")
            psB = psB_cm.__enter__()

            # E^T chunks (PE order: these first — only need e128 + ident)
            for k in range(4):
                etps = psA.tile([P, P], f32, tag="eps")
                nc.tensor.transpose(
                    etps[:], e128[:, 128 * k : 128 * (k + 1)], ident[:]
                )
                nc.scalar.copy(et[k][:], etps[:])
            for i, k in enumerate((2, 3)):
                nc.vector.tensor_scalar_mul(etn[i][:], et[k][:], -1.0)

            # GZ = F F^T - G G^T ; GX = F G^T + G F^T
            gzps = psG.tile([P, P], f32, tag="gram")
            nc.tensor.matmul(gzps[:], et[0][:], et[0][:], start=True, stop=False)
            nc.tensor.matmul(gzps[:], et[1][:], et[1][:], start=False, stop=False)
            nc.tensor.matmul(gzps[:], etn[0][:], et[2][:], start=False, stop=False)
            nc.tensor.matmul(gzps[:], etn[1][:], et[3][:], start=False, stop=True)
            nc.scalar.copy(gz[:], gzps[:])

            gxps = psG.tile([P, P], f32, tag="gram")
            nc.tensor.matmul(gxps[:], et[0][:], et[2][:], start=True, stop=False)
            nc.tensor.matmul(gxps[:], et[1][:], et[3][:], start=False, stop=False)
            nc.tensor.matmul(gxps[:], et[2][:], et[0][:], start=False, stop=False)
            nc.tensor.matmul(gxps[:], et[3][:], et[1][:], start=False, stop=True)
            nc.scalar.copy(gx[:], gxps[:])

            # cos/sin broadcast to 128 partitions (early on PE; evac on DVE
            # because ACT is busy with the norm squares around this time)
            csb_ps = psA.tile([P, 2 * C_OUT], f32, tag="eps")
            nc.tensor.matmul(
                csb_ps[:], ones1[:].bitcast(f32r), csrow[:], start=True, stop=True
            )
            nc.vector.tensor_copy(csmat[:], csb_ps[:])

            # squared norms per patch: transpose 128-patch blocks, square on
            # ACT, row-reduce on DVE (pipelined across the two engines)
            for ch in range(8):
                p2ps = psA.tile([P, P], f32, tag="eps")
                nc.tensor.transpose(
                    p2ps[:], pt[:, 128 * ch : 128 * (ch + 1)], ident[:]
                )
                scr = scrp.tile([P, P], f32, tag="scr")
                nc.scalar.activation(scr[:], p2ps[:], AF.Square)
                nc.vector.tensor_reduce(
                    n2all[:, ch : ch + 1], scr[:],
                    axis=mybir.AxisListType.X, op=ALU.add,
                )

            # main quadratic forms (start as soon as gz/gx + ptr are ready)
            qz = psB.tile([P, LP], f32, tag="qz")
            qx = psB.tile([P, LP], f32, tag="qx")
            for h in range(2):
                s = slice(512 * h, 512 * (h + 1))
                nc.tensor.matmul(qz[:, s], gz[:], ptr[:, s], start=True, stop=True)
            for h in range(2):
                s = slice(512 * h, 512 * (h + 1))
                nc.tensor.matmul(qx[:, s], gx[:], ptr[:, s], start=True, stop=True)

            mzn = sb.tile([P, LP], f32r)
            mxn = sb.tile([P, LP], f32r)
            for h in range(2):
                s = slice(512 * h, 512 * (h + 1))
                nc.vector.tensor_mul(mzn[:, s], pt[:, s], qz[:, s])
                nc.vector.tensor_mul(mxn[:, s], pt[:, s], qx[:, s])

            # 1/||p||^2 -> single transpose -> DRAM bounce broadcast to 16 rows
            nc.vector.tensor_scalar_max(n2all[:], n2all[:], 1e-24)
            nc.vector.reciprocal(invcol[:], n2all[:])
            invt8ps = psG.tile([8, P], f32, tag="gram")
            nc.tensor.transpose(invt8ps[:], invcol[:], ident[:])
            nc.scalar.copy(invt8[:], invt8ps[:])
            nc.scalar.dma_start(
                inv_dram[:].rearrange("o (c p) -> (o c) p", c=8, p=P), invt8[:]
            )
            inv_rep = _br.AP(inv_dram, 0, [[0, C_OUT], [1, LP]])
            nc.scalar.dma_start(invb[:], inv_rep)

            psB_cm.__exit__(None, None, None)

            # channel combine: out1[c,l] = cos_c*qZ[l] + sin_c*qX[l]
            psC = ps_stack.enter_context(
                tc.tile_pool(name="psC", bufs=1, space="PSUM")
            )
            outs = sb.tile([C_OUT, LP], f32)
            for h in range(2):
                s = slice(512 * h, 512 * (h + 1))
                out1 = psC.tile([C_OUT, 512], f32, name=f"out1_{h}", tag=f"o{h}")
                nc.tensor.matmul(
                    out1[:], csmat[:, 0:C_OUT], mzn[:, s],
                    start=True, stop=False,
                )
                nc.tensor.matmul(
                    out1[:], csmat[:, C_OUT : 2 * C_OUT], mxn[:, s],
                    start=False, stop=True,
                )
                # divide by ||p||^2 while evacuating PSUM, pipelined per half
                nc.vector.tensor_mul(outs[:, s], invb[:, s], out1[:])
                nc.sync.dma_start(out_ext[:, s], outs[:, s])

            if dbg:
                for nm, t in [
                    ("d_pt", pt), ("d_e128", e128), ("d_csrow", csrow),
                    ("d_gz", gz), ("d_gx", gx), ("d_n2all", n2all),
                    ("d_invb", invb), ("d_mzn", mzn), ("d_mxn", mxn),
                ]:
                    nc.sync.dma_start(
                        dbg_ext[nm][:], t[:].bitcast(f32)
                    )


    nc.compile()
    return nc


def kernel(**inputs):
    from concourse.bass_utils import run_bass_kernel_spmd

    x = np.ascontiguousarray(np.asarray(inputs["x"], dtype=np.float32))
    theta = np.ascontiguousarray(np.asarray(inputs["theta"], dtype=np.float32))
    ent = np.ascontiguousarray(
        np.asarray(inputs["entangle_matrix"], dtype=np.float32)
    )

    if "nc" not in _CACHE:
        _CACHE["nc"] = _build_nc()
    nc = _CACHE["nc"]

    in_maps = [
        {"x": np.ascontiguousarray(x[b]), "theta": theta, "entangle": ent}
        for b in range(B)
    ]
    res = run_bass_kernel_spmd(nc, in_maps, core_ids=list(range(B)))
    out = np.stack([res.results[b]["out"][:, :L_OUT] for b in range(B)], axis=0)
    return np.ascontiguousarray(out.astype(np.float32))


# revision 24
# speedup vs baseline: 1.5556x; 1.0886x over previous
"""DenseQConv1D Trainium2 kernel.

Math: the reference computes, per output channel c and patch p (128-dim im2col
column of x, normalized):
    out[c,p] = sum_e sign(e) * (s_p^T (E @ R_c)[:128,:])_e^2
with R_c = kron of 9 RY(theta[c,q]) rotations and sign(e) = Z on the MSB qubit.
Because every RY factor is orthogonal and the measurement only touches qubit 0,
    R_c S R_c^T = kron([[cos t, sin t], [sin t, -cos t]], I_256),  t = theta[c,0]
so with E128 = E[:128,:], F = E128[:,:256], G = E128[:,256:]:
    GZ = F F^T - G G^T,  GX = F G^T + G F^T   (both 128x128, theta-independent)
    out[c,p] = (cos t_c * p^T GZ p + sin t_c * p^T GX p) / ||p||^2
This is exact for any entangle matrix / theta (validated: rel err ~9e-7 in
fp32; the shipped kernel uses fp32r matmuls, rel err ~1e-4).

Sharding: batch dimension across the 8 cores (core b computes x[b]); theta and
the entangle matrix are replicated. Everything (GZ/GX construction, im2col
quadratic forms, cos/sin, normalization) runs on-device.
"""

import math
from contextlib import ExitStack as _ExitStack

import numpy as np

B = 8
C_IN = 16
C_OUT = 16
L = 1024
K = 8
L_OUT = L - K + 1  # 1017
LP = 1024  # padded patch count per core (cols 1017:1024 are dummy)
P = 128  # patch vector length = C_IN*K = partitions

_CACHE = {}


def _build_nc(dbg=False):
    import bass_rust as _br
    import concourse.bacc as bacc
    import concourse.mybir as mybir
    import concourse.tile as tile
    from concourse import masks

    f32 = mybir.dt.float32
    f32r = mybir.dt.float32r
    AF = mybir.ActivationFunctionType
    ALU = mybir.AluOpType
    nc = bacc.Bacc("TRN2", target_bir_lowering=False, debug=False)

    x_ext = nc.declare_dram_parameter("x", [C_IN, L], f32, isOutput=False)
    th_ext = nc.declare_dram_parameter("theta", [C_OUT, 9], f32, isOutput=False)
    e_ext = nc.declare_dram_parameter("entangle", [512, 512], f32, isOutput=False)
    out_ext = nc.declare_dram_parameter("out", [C_OUT, LP], f32, isOutput=True)
    inv_dram = nc.dram_tensor("inv_scratch", [1, LP], f32)
    dbg_ext = {}
    if dbg:
        for nm, shp in [
            ("d_pt", [P, LP]), ("d_e128", [P, 512]), ("d_csrow", [1, 2 * C_OUT]),
            ("d_gz", [P, P]), ("d_gx", [P, P]),
            ("d_n2all", [P, 8]), ("d_invb", [C_OUT, LP]),
            ("d_mzn", [P, LP]), ("d_mxn", [P, LP]),
        ]:
            dbg_ext[nm] = nc.declare_dram_parameter(nm, shp, f32, isOutput=True)

    with tile.TileContext(nc) as tc, tc.tile_pool(name="const", bufs=1) as const, \
            tc.tile_pool(name="sb", bufs=1) as sb, \
            tc.tile_pool(name="scr", bufs=2) as scrp:
        ident = const.tile([P, P], f32)
        masks.make_identity(nc, ident[:])
        ones1 = const.tile([1, P], f32)
        nc.vector.memset(ones1[:], 1.0)

        # ---- loads: theta (tiny) first, then E halves, then x halves;
        # interleaved across the two HWDGE rings so E lands first ----
        th = sb.tile([1, C_OUT], f32)
        nc.sync.dma_start(th[:], th_ext[:, 0:1].rearrange("p o -> o p"))

        # E rows 0..127, row-permuted (c j) -> (j c); partition halves on
        # the two rings
        e128 = sb.tile([P, 512], f32)
        e_perm0 = _br.AP(e_ext, 0, [[512, 4], [512 * K, C_IN], [1, 512]])
        e_perm1 = _br.AP(e_ext, 4 * 512, [[512, 4], [512 * K, C_IN], [1, 512]])
        nc.sync.dma_start(e128[0:64, :], e_perm0)
        nc.scalar.dma_start(e128[64:P, :], e_perm1)

        # PT[j*16+c, l] = x[c, l+j]: im2col via overlapping-window APs
        pt = sb.tile([P, LP], f32)
        x_win0 = _br.AP(x_ext, 0, [[1, 4], [L, C_IN], [1, L_OUT]])
        x_win1 = _br.AP(x_ext, 4, [[1, 4], [L, C_IN], [1, L_OUT]])
        nc.sync.dma_start(pt[0:64, 0:L_OUT], x_win0)
        nc.scalar.dma_start(pt[64:P, 0:L_OUT], x_win1)
        nc.vector.memset(pt[:, L_OUT:LP], 1.0)

        # preload both ACT PWP tables with dummy (1,1) ops so no table load
        # lands on the critical path later
        bias_zero = const.tile([1, 1], f32)
        nc.scalar.memzero(bias_zero[:])
        bias_half_pi = const.tile([1, 1], f32)
        nc.scalar.activation(
            bias_half_pi[:], bias_zero[:], AF.Copy, bias=math.pi / 2.0, scale=1.0
        )
        tdum = const.tile([1, 1], f32)
        nc.scalar.activation(tdum[:], bias_zero[:], AF.Sin, bias=bias_zero[:])
        nc.scalar.activation(tdum[:], bias_zero[:], AF.Square, bias=bias_zero[:])

        # |t| and sign(t) on DVE (cheap, keeps ACT free)
        csrow = sb.tile([1, 2 * C_OUT], f32r)
        ta = sb.tile([1, C_OUT], f32)
        nc.vector.scalar_tensor_tensor(
            ta[:], th[:], -1.0, th[:], op0=ALU.mult, op1=ALU.max
        )
        tsgn = sb.tile([1, C_OUT], f32)
        nc.vector.tensor_scalar(tsgn[:], th[:], 0.0, None, op0=ALU.is_gt)
        nc.vector.tensor_scalar(
            tsgn[:], tsgn[:], 2.0, 1.0, op0=ALU.mult, op1=ALU.subtract
        )

        # fp32r copy of pt for the PE, on DVE (2x single-src mode)
        ptr = sb.tile([P, LP], f32r)
        nc.vector.tensor_copy(ptr[:], pt[:])

        csmat = sb.tile([P, 2 * C_OUT], f32r)
        et = [sb.tile([P, P], f32r, name=f"et{k}", tag=f"et{k}") for k in range(4)]
        etn = [sb.tile([P, P], f32r, name=f"etn{k}", tag=f"etn{k}") for k in range(2)]
        gz = sb.tile([P, P], f32r)
        gx = sb.tile([P, P], f32r)
        n2all = sb.tile([P, 8], f32)
        invcol = sb.tile([P, 8], f32)
        invt8 = sb.tile([8, P], f32)
        invb = sb.tile([C_OUT, LP], f32)

        # PSUM pools in strict stack order: psA+psG (4 banks, outer, live
        # throughout), psB (qz/qx, 4 banks) closed before psC (out1) opens.
        with _ExitStack() as ps_stack:
            psA = ps_stack.enter_context(
                tc.tile_pool(name="psA", bufs=2, space="PSUM")
            )
            psG = ps_stack.enter_context(
                tc.tile_pool(name="psG", bufs=2, space="PSUM")
            )
            psB_cm = tc.tile_pool(name="psB", bufs=1, space="PSUM")
            psB = psB_cm.__enter__()

            # E^T chunks (PE order: these first — only need e128 + ident)
            for k in range(4):
                etps = psA.tile([P, P], f32, tag="eps")
                nc.tensor.transpose(
                    etps[:], e128[:, 128 * k : 128 * (k + 1)], ident[:]
                )
                nc.scalar.copy(et[k][:], etps[:])
            for i, k in enumerate((2, 3)):
                nc.vector.tensor_scalar_mul(etn[i][:], et[k][:], -1.0)

            # patch-block transposes for the norms (PE, before gram so the
            # ACT squares can start early)
            p2list = []
            for ch in range(8):
                p2ps = psA.tile([P, P], f32, name=f"p2ps{ch}", tag="eps")
                nc.tensor.transpose(
                    p2ps[:], pt[:, 128 * ch : 128 * (ch + 1)], ident[:]
                )
                p2list.append(p2ps)

            # GZ = F F^T - G G^T ; GX = F G^T + G F^T
            gzps = psG.tile([P, P], f32, tag="gram")
            nc.tensor.matmul(gzps[:], et[0][:], et[0][:], start=True, stop=False)
            nc.tensor.matmul(gzps[:], et[1][:], et[1][:], start=False, stop=False)
            nc.tensor.matmul(gzps[:], etn[0][:], et[2][:], start=False, stop=False)
            nc.tensor.matmul(gzps[:], etn[1][:], et[3][:], start=False, stop=True)

            gxps = psG.tile([P, P], f32, tag="gram")
            nc.tensor.matmul(gxps[:], et[0][:], et[2][:], start=True, stop=False)
            nc.tensor.matmul(gxps[:], et[1][:], et[3][:], start=False, stop=False)
            nc.tensor.matmul(gxps[:], et[2][:], et[0][:], start=False, stop=False)
            nc.tensor.matmul(gxps[:], et[3][:], et[1][:], start=False, stop=True)

            # ACT: squares (feed the 1/n2 chain) then gz/gx evacuations;
            # DVE row-reduces pipelined behind the squares
            nc.vector.tensor_copy(gz[:], gzps[:])
            nc.vector.tensor_copy(gx[:], gxps[:])
            for ch in range(8):
                scr = scrp.tile([P, P], f32, tag="scr")
                nc.scalar.activation(scr[:], p2list[ch][:], AF.Square)
                nc.vector.tensor_reduce(
                    n2all[:, ch : ch + 1], scr[:],
                    axis=mybir.AxisListType.X, op=ALU.add,
                )

            # 1/||p||^2 -> single transpose -> DRAM bounce broadcast to 16 rows
            nc.vector.tensor_scalar_max(n2all[:], n2all[:], 1e-24)
            nc.vector.reciprocal(invcol[:], n2all[:])
            invt8ps = psG.tile([8, P], f32, tag="gram")
            nc.tensor.transpose(invt8ps[:], invcol[:], ident[:])
            nc.scalar.copy(invt8[:], invt8ps[:])
            nc.scalar.dma_start(
                inv_dram[:].rearrange("o (c p) -> (o c) p", c=8, p=P), invt8[:]
            )
            inv_rep = _br.AP(inv_dram, 0, [[0, C_OUT], [1, LP]])
            nc.scalar.dma_start(invb[:], inv_rep)

            # main quadratic forms
            qz = psB.tile([P, LP], f32, tag="qz")
            qx = psB.tile([P, LP], f32, tag="qx")
            for h in range(2):
                s = slice(512 * h, 512 * (h + 1))
                nc.tensor.matmul(qz[:, s], gz[:], ptr[:, s], start=True, stop=True)
            for h in range(2):
                s = slice(512 * h, 512 * (h + 1))
                nc.tensor.matmul(qx[:, s], gx[:], ptr[:, s], start=True, stop=True)

            mzn = sb.tile([P, LP], f32r)
            mxn = sb.tile([P, LP], f32r)
            for h in range(2):
                s = slice(512 * h, 512 * (h + 1))
                nc.vector.tensor_mul(mzn[:, s], pt[:, s], qz[:, s])
                nc.vector.tensor_mul(mxn[:, s], pt[:, s], qx[:, s])

            # trig: u = sin(|t|/2), v = cos(|t|/2); cos t = 1-2u^2,
            # sin t = sign(t)*2uv  (ACT is free once the squares are done)
            u = sb.tile([1, C_OUT], f32)
            nc.scalar.activation(u[:], ta[:], AF.Sin, bias=bias_zero[:], scale=0.5)
            v = sb.tile([1, C_OUT], f32)
            nc.scalar.activation(
                v[:], ta[:], AF.Sin, bias=bias_half_pi[:], scale=-0.5
            )
            u2 = sb.tile([1, C_OUT], f32)
            nc.scalar.activation(
                u2[:], u[:], AF.Square, bias=bias_zero[:], scale=math.sqrt(2.0)
            )
            nc.scalar.activation(
                csrow[:, 0:C_OUT], u2[:], AF.Copy, bias=1.0, scale=-1.0
            )
            uv = sb.tile([1, C_OUT], f32)
            nc.vector.tensor_mul(uv[:], u[:], v[:])
            nc.vector.scalar_tensor_tensor(
                csrow[:, C_OUT : 2 * C_OUT], uv[:], 2.0, tsgn[:],
                op0=ALU.mult, op1=ALU.mult,
            )

            # cos/sin broadcast to 128 partitions; evac on DVE
            csb_ps = psA.tile([P, 2 * C_OUT], f32, tag="eps")
            nc.tensor.matmul(
                csb_ps[:], ones1[:].bitcast(f32r), csrow[:], start=True, stop=True
            )
            nc.vector.tensor_copy(csmat[:], csb_ps[:])

            psB_cm.__exit__(None, None, None)

            # channel combine: out1[c,l] = cos_c*qZ[l] + sin_c*qX[l]
            psC = ps_stack.enter_context(
                tc.tile_pool(name="psC", bufs=1, space="PSUM")
            )
            outs = sb.tile([C_OUT, LP], f32)
            for h in range(2):
                s = slice(512 * h, 512 * (h + 1))
                out1 = psC.tile([C_OUT, 512], f32, name=f"out1_{h}", tag=f"o{h}")
                nc.tensor.matmul(
                    out1[:], csmat[:, 0:C_OUT], mzn[:, s],
                    start=True, stop=False,
                )
                nc.tensor.matmul(
                    out1[:], csmat[:, C_OUT : 2 * C_OUT], mxn[:, s],
                    start=False, stop=True,
                )
                # divide by ||p||^2 while evacuating PSUM, pipelined per half
                nc.vector.tensor_mul(outs[:, s], invb[:, s], out1[:])
                nc.sync.dma_start(out_ext[:, s], outs[:, s])

            if dbg:
                for nm, t in [
                    ("d_pt", pt), ("d_e128", e128), ("d_csrow", csrow),
                    ("d_gz", gz), ("d_gx", gx), ("d_n2all", n2all),
                    ("d_invb", invb), ("d_mzn", mzn), ("d_mxn", mxn),
                ]:
                    nc.sync.dma_start(
                        dbg_ext[nm][:], t[:].bitcast(f32)
                    )


    nc.compile()
    return nc


def kernel(**inputs):
    from concourse.bass_utils import run_bass_kernel_spmd

    x = np.ascontiguousarray(np.asarray(inputs["x"], dtype=np.float32))
    theta = np.ascontiguousarray(np.asarray(inputs["theta"], dtype=np.float32))
    ent = np.ascontiguousarray(
        np.asarray(inputs["entangle_matrix"], dtype=np.float32)
    )

    if "nc" not in _CACHE:
        _CACHE["nc"] = _build_nc()
    nc = _CACHE["nc"]

    in_maps = [
        {"x": np.ascontiguousarray(x[b]), "theta": theta, "entangle": ent}
        for b in range(B)
    ]
    res = run_bass_kernel_spmd(nc, in_maps, core_ids=list(range(B)))
    out = np.stack([res.results[b]["out"][:, :L_OUT] for b in range(B)], axis=0)
    return np.ascontiguousarray(out.astype(np.float32))


# revision 25
# speedup vs baseline: 1.6646x; 1.0701x over previous
"""DenseQConv1D Trainium2 kernel.

Math: the reference computes, per output channel c and patch p (128-dim im2col
column of x, normalized):
    out[c,p] = sum_e sign(e) * (s_p^T (E @ R_c)[:128,:])_e^2
with R_c = kron of 9 RY(theta[c,q]) rotations and sign(e) = Z on the MSB qubit.
Because every RY factor is orthogonal and the measurement only touches qubit 0,
    R_c S R_c^T = kron([[cos t, sin t], [sin t, -cos t]], I_256),  t = theta[c,0]
so with E128 = E[:128,:], F = E128[:,:256], G = E128[:,256:]:
    GZ = F F^T - G G^T,  GX = F G^T + G F^T   (both 128x128, theta-independent)
    out[c,p] = (cos t_c * p^T GZ p + sin t_c * p^T GX p) / ||p||^2
This is exact for any entangle matrix / theta (validated: rel err ~9e-7 in
fp32; the shipped kernel uses fp32r matmuls, rel err ~1e-4).

Sharding: batch dimension across the 8 cores (core b computes x[b]); theta and
the entangle matrix are replicated. Everything (GZ/GX construction, im2col
quadratic forms, cos/sin, normalization) runs on-device.
"""

import math
from contextlib import ExitStack as _ExitStack

import numpy as np

B = 8
C_IN = 16
C_OUT = 16
L = 1024
K = 8
L_OUT = L - K + 1  # 1017
LP = 1024  # padded patch count per core (cols 1017:1024 are dummy)
P = 128  # patch vector length = C_IN*K = partitions

_CACHE = {}


def _build_nc(dbg=False):
    import bass_rust as _br
    import concourse.bacc as bacc
    import concourse.mybir as mybir
    import concourse.tile as tile
    from concourse import masks

    f32 = mybir.dt.float32
    f32r = mybir.dt.float32r
    AF = mybir.ActivationFunctionType
    ALU = mybir.AluOpType
    nc = bacc.Bacc("TRN2", target_bir_lowering=False, debug=False)

    x_ext = nc.declare_dram_parameter("x", [C_IN, L], f32, isOutput=False)
    th_ext = nc.declare_dram_parameter("theta", [C_OUT, 9], f32, isOutput=False)
    e_ext = nc.declare_dram_parameter("entangle", [512, 512], f32, isOutput=False)
    out_ext = nc.declare_dram_parameter("out", [C_OUT, LP], f32, isOutput=True)
    inv_dram = nc.dram_tensor("inv_scratch", [1, LP], f32)
    dbg_ext = {}
    if dbg:
        for nm, shp in [
            ("d_pt", [P, LP]), ("d_e128", [P, 512]), ("d_csrow", [1, 2 * C_OUT]),
            ("d_gz", [P, P]), ("d_gx", [P, P]),
            ("d_n2all", [P, 8]), ("d_invb", [C_OUT, LP]),
            ("d_mzn", [P, LP]), ("d_mxn", [P, LP]),
        ]:
            dbg_ext[nm] = nc.declare_dram_parameter(nm, shp, f32, isOutput=True)

    with tile.TileContext(nc) as tc, tc.tile_pool(name="const", bufs=1) as const, \
            tc.tile_pool(name="sb", bufs=1) as sb, \
            tc.tile_pool(name="scr", bufs=2) as scrp:
        ident = const.tile([P, P], f32)
        masks.make_identity(nc, ident[:])
        ones1 = const.tile([1, P], f32)
        nc.vector.memset(ones1[:], 1.0)

        # ---- loads: theta (tiny) first, then E halves, then x halves;
        # interleaved across the two HWDGE rings so E lands first ----
        th = sb.tile([1, C_OUT], f32)
        nc.sync.dma_start(th[:], th_ext[:, 0:1].rearrange("p o -> o p"))

        # E rows 0..127, row-permuted (c j) -> (j c); partition halves on
        # the two rings
        e128 = sb.tile([P, 512], f32)
        e_perm0 = _br.AP(e_ext, 0, [[512, 4], [512 * K, C_IN], [1, 512]])
        e_perm1 = _br.AP(e_ext, 4 * 512, [[512, 4], [512 * K, C_IN], [1, 512]])
        nc.sync.dma_start(e128[0:64, :], e_perm0)
        nc.scalar.dma_start(e128[64:P, :], e_perm1)

        # PT[j*16+c, l] = x[c, l+j]: im2col via overlapping-window APs
        pt = sb.tile([P, LP], f32)
        x_win0 = _br.AP(x_ext, 0, [[1, 4], [L, C_IN], [1, L_OUT]])
        x_win1 = _br.AP(x_ext, 4, [[1, 4], [L, C_IN], [1, L_OUT]])
        nc.sync.dma_start(pt[0:64, 0:L_OUT], x_win0)
        nc.scalar.dma_start(pt[64:P, 0:L_OUT], x_win1)
        nc.vector.memset(pt[:, L_OUT:LP], 1.0)

        # preload both ACT PWP tables with dummy (1,1) ops so no table load
        # lands on the critical path later
        bias_zero = const.tile([1, 1], f32)
        nc.scalar.memzero(bias_zero[:])
        bias_half_pi = const.tile([1, 1], f32)
        nc.scalar.activation(
            bias_half_pi[:], bias_zero[:], AF.Copy, bias=math.pi / 2.0, scale=1.0
        )
        tdum = const.tile([1, 1], f32)
        nc.scalar.activation(tdum[:], bias_zero[:], AF.Sin, bias=bias_zero[:])
        nc.scalar.activation(tdum[:], bias_zero[:], AF.Square, bias=bias_zero[:])

        # |t| and sign(t) on DVE (cheap, keeps ACT free)
        csrow = sb.tile([1, 2 * C_OUT], f32r)
        ta = sb.tile([1, C_OUT], f32)
        nc.vector.scalar_tensor_tensor(
            ta[:], th[:], -1.0, th[:], op0=ALU.mult, op1=ALU.max
        )
        tsgn = sb.tile([1, C_OUT], f32)
        nc.vector.tensor_scalar(tsgn[:], th[:], 0.0, None, op0=ALU.is_gt)
        nc.vector.tensor_scalar(
            tsgn[:], tsgn[:], 2.0, 1.0, op0=ALU.mult, op1=ALU.subtract
        )

        # fp32r copy of pt for the PE, on DVE (2x single-src mode)
        ptr = sb.tile([P, LP], f32r)
        nc.vector.tensor_copy(ptr[:], pt[:])

        csmat = sb.tile([P, 2 * C_OUT], f32r)
        et = [sb.tile([P, P], f32r, name=f"et{k}", tag=f"et{k}") for k in range(4)]
        etn = [sb.tile([P, P], f32r, name=f"etn{k}", tag=f"etn{k}") for k in range(2)]
        gz = sb.tile([P, P], f32r)
        gx = sb.tile([P, P], f32r)
        n2all = sb.tile([P, 8], f32)
        invcol = sb.tile([P, 8], f32)
        invt8 = sb.tile([8, P], f32)
        invb = sb.tile([C_OUT, LP], f32)

        # PSUM pools in strict stack order: psA+psG (4 banks, outer, live
        # throughout), psB (qz/qx, 4 banks) closed before psC (out1) opens.
        with _ExitStack() as ps_stack:
            psA = ps_stack.enter_context(
                tc.tile_pool(name="psA", bufs=2, space="PSUM")
            )
            psG = ps_stack.enter_context(
                tc.tile_pool(name="psG", bufs=2, space="PSUM")
            )
            psB_cm = tc.tile_pool(name="psB", bufs=1, space="PSUM")
            psB = psB_cm.__enter__()

            # E^T chunks (PE order: these first — only need e128 + ident)
            for k in range(4):
                etps = psA.tile([P, P], f32, tag="eps")
                nc.tensor.transpose(
                    etps[:], e128[:, 128 * k : 128 * (k + 1)], ident[:]
                )
                nc.scalar.copy(et[k][:], etps[:])
            for i, k in enumerate((2, 3)):
                nc.vector.tensor_scalar_mul(etn[i][:], et[k][:], -1.0)

            # patch-block transposes for the norms (PE, before gram so the
            # ACT squares can start early); batched 4 chunks per PSUM bank
            p2list = []
            for b in range(2):
                p2big = psA.tile([P, 512], f32, name=f"p2big{b}", tag="p2b")
                for q in range(4):
                    ch = 4 * b + q
                    nc.tensor.transpose(
                        p2big[:, 128 * q : 128 * (q + 1)],
                        pt[:, 128 * ch : 128 * (ch + 1)], ident[:],
                    )
                p2list.append(p2big)

            # GZ = F F^T - G G^T ; GX = F G^T + G F^T
            gzps = psG.tile([P, P], f32, tag="gram")
            nc.tensor.matmul(gzps[:], et[0][:], et[0][:], start=True, stop=False)
            nc.tensor.matmul(gzps[:], et[1][:], et[1][:], start=False, stop=False)
            nc.tensor.matmul(gzps[:], etn[0][:], et[2][:], start=False, stop=False)
            nc.tensor.matmul(gzps[:], etn[1][:], et[3][:], start=False, stop=True)

            gxps = psG.tile([P, P], f32, tag="gram")
            nc.tensor.matmul(gxps[:], et[0][:], et[2][:], start=True, stop=False)
            nc.tensor.matmul(gxps[:], et[1][:], et[3][:], start=False, stop=False)
            nc.tensor.matmul(gxps[:], et[2][:], et[0][:], start=False, stop=False)
            nc.tensor.matmul(gxps[:], et[3][:], et[1][:], start=False, stop=True)

            # ACT: squares (feed the 1/n2 chain) then gz/gx evacuations;
            # DVE row-reduces pipelined behind the squares
            nc.vector.tensor_copy(gz[:], gzps[:])
            nc.vector.tensor_copy(gx[:], gxps[:])
            for b in range(2):
                scr = scrp.tile([P, 512], f32, tag="scr")
                nc.scalar.activation(scr[:], p2list[b][:], AF.Square)
                nc.vector.tensor_reduce(
                    n2all[:, 4 * b : 4 * (b + 1)],
                    scr[:].rearrange("p (c i) -> p c i", c=4, i=P),
                    axis=mybir.AxisListType.X, op=ALU.add,
                )

            # 1/||p||^2 -> single transpose -> DRAM bounce broadcast to 16 rows
            nc.vector.tensor_scalar_max(n2all[:], n2all[:], 1e-24)
            nc.vector.reciprocal(invcol[:], n2all[:])
            invt8ps = psA.tile([8, P], f32, tag="eps")
            nc.tensor.transpose(invt8ps[:], invcol[:], ident[:])
            nc.scalar.copy(invt8[:], invt8ps[:])
            nc.scalar.dma_start(
                inv_dram[:].rearrange("o (c p) -> (o c) p", c=8, p=P), invt8[:]
            )
            inv_rep = _br.AP(inv_dram, 0, [[0, C_OUT], [1, LP]])
            nc.scalar.dma_start(invb[:], inv_rep)

            # main quadratic forms; per-half PSUM tiles in a 2-slot tag
            mzn = sb.tile([P, LP], f32r)
            mxn = sb.tile([P, LP], f32r)
            for h in range(2):
                s = slice(512 * h, 512 * (h + 1))
                qzh = psB.tile([P, 512], f32, name=f"qz{h}", tag="q")
                nc.tensor.matmul(qzh[:], gz[:], ptr[:, s], start=True, stop=True)
                qxh = psB.tile([P, 512], f32, name=f"qx{h}", tag="q")
                nc.tensor.matmul(qxh[:], gx[:], ptr[:, s], start=True, stop=True)
                nc.vector.tensor_mul(mzn[:, s], pt[:, s], qzh[:])
                nc.vector.tensor_mul(mxn[:, s], pt[:, s], qxh[:])

            # trig: u = sin(|t|/2), v = cos(|t|/2); cos t = 1-2u^2,
            # sin t = sign(t)*2uv  (ACT is free once the squares are done)
            u = sb.tile([1, C_OUT], f32)
            nc.scalar.activation(u[:], ta[:], AF.Sin, bias=bias_zero[:], scale=0.5)
            v = sb.tile([1, C_OUT], f32)
            nc.scalar.activation(
                v[:], ta[:], AF.Sin, bias=bias_half_pi[:], scale=-0.5
            )
            u2 = sb.tile([1, C_OUT], f32)
            nc.scalar.activation(
                u2[:], u[:], AF.Square, bias=bias_zero[:], scale=math.sqrt(2.0)
            )
            nc.scalar.activation(
                csrow[:, 0:C_OUT], u2[:], AF.Copy, bias=1.0, scale=-1.0
            )
            uv = sb.tile([1, C_OUT], f32)
            nc.vector.tensor_mul(uv[:], u[:], v[:])
            nc.vector.scalar_tensor_tensor(
                csrow[:, C_OUT : 2 * C_OUT], uv[:], 2.0, tsgn[:],
                op0=ALU.mult, op1=ALU.mult,
            )

            # cos/sin broadcast to 128 partitions; evac on DVE
            csb_ps = psA.tile([P, 2 * C_OUT], f32, tag="eps")
            nc.tensor.matmul(
                csb_ps[:], ones1[:].bitcast(f32r), csrow[:], start=True, stop=True
            )
            nc.vector.tensor_copy(csmat[:], csb_ps[:])

            psB_cm.__exit__(None, None, None)

            # channel combine: out1[c,l] = cos_c*qZ[l] + sin_c*qX[l]
            psC = ps_stack.enter_context(
                tc.tile_pool(name="psC", bufs=1, space="PSUM")
            )
            outs = sb.tile([C_OUT, LP], f32)
            for h in range(2):
                s = slice(512 * h, 512 * (h + 1))
                out1 = psC.tile([C_OUT, 512], f32, name=f"out1_{h}", tag=f"o{h}")
                nc.tensor.matmul(
                    out1[:], csmat[:, 0:C_OUT], mzn[:, s],
                    start=True, stop=False,
                )
                nc.tensor.matmul(
                    out1[:], csmat[:, C_OUT : 2 * C_OUT], mxn[:, s],
                    start=False, stop=True,
                )
                # divide by ||p||^2 while evacuating PSUM, pipelined per half
                nc.vector.tensor_mul(outs[:, s], invb[:, s], out1[:])
                nc.sync.dma_start(out_ext[:, s], outs[:, s])

            if dbg:
                for nm, t in [
                    ("d_pt", pt), ("d_e128", e128), ("d_csrow", csrow),
                    ("d_gz", gz), ("d_gx", gx), ("d_n2all", n2all),
                    ("d_invb", invb), ("d_mzn", mzn), ("d_mxn", mxn),
                ]:
                    nc.sync.dma_start(
                        dbg_ext[nm][:], t[:].bitcast(f32)
                    )


    nc.compile()
    return nc


def kernel(**inputs):
    from concourse.bass_utils import run_bass_kernel_spmd

    x = np.ascontiguousarray(np.asarray(inputs["x"], dtype=np.float32))
    theta = np.ascontiguousarray(np.asarray(inputs["theta"], dtype=np.float32))
    ent = np.ascontiguousarray(
        np.asarray(inputs["entangle_matrix"], dtype=np.float32)
    )

    if "nc" not in _CACHE:
        _CACHE["nc"] = _build_nc()
    nc = _CACHE["nc"]

    in_maps = [
        {"x": np.ascontiguousarray(x[b]), "theta": theta, "entangle": ent}
        for b in range(B)
    ]
    res = run_bass_kernel_spmd(nc, in_maps, core_ids=list(range(B)))
    out = np.stack([res.results[b]["out"][:, :L_OUT] for b in range(B)], axis=0)
    return np.ascontiguousarray(out.astype(np.float32))
